# revision 7
# baseline (speedup 1.0000x reference)
"""Conv2d 3x3 (stride 1, pad 1) as 9 shifted matmuls on TRN2, data-parallel
over batch across 8 NeuronCores.

Full shapes: img [32,128,112,112] f32, weight [256,128,3,3] f32, bias [256] f32
-> out [32,256,112,112] f32.

Per core: 4 images. C_in=128 is the contraction/partition dim. The image
lives in SBUF as a zero-padded [128, 114, 114] fp32r buffer; each PSUM tile
covers 4 output rows [128, 4, 112] and accumulates 9 matmuls (one per filter
tap) reading 3D-strided slices of the padded buffer. Weights are
host-transposed to [C_in, 9, 2, 128] so lhsT tiles are direct slices.
"""

import os
import sys

sys.path.insert(0, "/opt/trn_rl_repo")

import numpy as np

N_CORES = 8
N, C_IN, H, W = 32, 128, 112, 112
C_OUT, KH, KW = 256, 3, 3
PER_CORE = N // N_CORES           # 4 images
HP, WP = H + 2, W + 2             # padded 114 x 114
RPC = 4                           # output rows per PSUM tile
NCHUNK = H // RPC                 # 28 chunks
MT = C_OUT // 128                 # 2 C_out tiles

# matmul input dtype: "f32r" (fp32 relaxed, 1 cyc/row at N>=256) or "bf16"
MM_DTYPE = os.environ.get("CONV_MM_DTYPE", "f32r")

_CACHED = {}


def _build(repeat: int = 1):
    import contextlib
    import concourse.tile as tile
    import concourse.mybir as mybir
    from concourse import bacc

    F32 = mybir.dt.float32
    CDT = mybir.dt.float32r if MM_DTYPE == "f32r" else mybir.dt.bfloat16

    nc = bacc.Bacc("TRN2", target_bir_lowering=False, debug=False)
    img_d = nc.dram_tensor("img", [PER_CORE, C_IN, H, W], F32,
                           kind="ExternalInput").ap()
    wt_d = nc.dram_tensor("wt", [C_IN, KH * KW, MT, 128], F32,
                          kind="ExternalInput").ap()
    bias_d = nc.dram_tensor("bias", [128, MT], F32, kind="ExternalInput").ap()
    out_d = nc.dram_tensor("out", [PER_CORE, C_OUT, H, W], F32,
                           kind="ExternalOutput").ap()

    with tile.TileContext(nc) as tc:
        with tc.tile_pool(name="const", bufs=1) as const_pool, \
             tc.tile_pool(name="imgpad", bufs=2) as imgpad_pool, \
             tc.tile_pool(name="ldchunk", bufs=4) as ld_pool, \
             tc.tile_pool(name="outsb", bufs=4) as out_pool, \
             tc.tile_pool(name="psum", bufs=8, space="PSUM") as psum_pool:

            # ---- constants: weights (converted to CDT) + bias ----
            wt_f32 = const_pool.tile([C_IN, KH * KW, MT, 128], F32)
            nc.sync.dma_start(wt_f32[:], wt_d[:])
            wt_c = const_pool.tile([C_IN, KH * KW, MT, 128], CDT)
            nc.vector.tensor_copy(out=wt_c[:], in_=wt_f32[:])
            bias_sb = const_pool.tile([128, MT], F32)
            nc.sync.dma_start(bias_sb[:], bias_d[:])
            zrow = const_pool.tile([C_IN, WP], F32)
            nc.vector.memset(zrow[:], 0.0)

            # repeat>1 wraps the whole body in a hardware loop for timing
            # amplification (identical work each iteration, same output).
            loop_ctx = tc.For_i(0, repeat, 1) if repeat > 1 \
                else contextlib.nullcontext()
            with loop_ctx:
              for i in range(PER_CORE):
                # ---- load + convert image i into padded CDT buffer ----
                imgp = imgpad_pool.tile([C_IN, HP, WP], CDT)
                # fp32r tiles cannot be memset; zero the pads via DVE copy
                # from an f32 zero row (a legal fp32r-rounding producer).
                nc.vector.tensor_copy(out=imgp[:, 0, :], in_=zrow[:])
                nc.vector.tensor_copy(out=imgp[:, HP - 1, :], in_=zrow[:])
                nc.vector.tensor_copy(out=imgp[:, 0:HP - 1, WP - 1],
                                      in_=zrow[:, 0:HP - 1])
                nc.vector.tensor_copy(out=imgp[:, 1:HP, 0],
                                      in_=zrow[:, 0:HP - 1])
                for c in range(NCHUNK):
                    ld = ld_pool.tile([C_IN, RPC, W], F32)
                    nc.sync.dma_start(ld[:], img_d[i, :, c * RPC:(c + 1) * RPC, :])
                    nc.vector.tensor_copy(
                        out=imgp[:, 1 + c * RPC: 1 + (c + 1) * RPC, 1:1 + W],
                        in_=ld[:])

                # ---- conv: 2 C_out tiles x 28 row-chunks x 9 taps ----
                for mt in range(MT):
                    for c in range(NCHUNK):
                        ps = psum_pool.tile([128, RPC, W], F32)
                        t = 0
                        for ky in range(KH):
                            for kx in range(KW):
                                nc.tensor.matmul(
                                    ps[:],
                                    lhsT=wt_c[:, ky * KW + kx, mt, :],
                                    rhs=imgp[:, c * RPC + ky: c * RPC + ky + RPC,
                                             kx: kx + W],
                                    start=(t == 0), stop=(t == KH * KW - 1),
                                )
                                t += 1
                        osb = out_pool.tile([128, RPC, W], F32)
                        nc.vector.tensor_scalar_add(osb[:], ps[:],
                                                    bias_sb[:, mt:mt + 1])
                        nc.sync.dma_start(
                            out_d[i, mt * 128:(mt + 1) * 128,
                                  c * RPC:(c + 1) * RPC, :],
                            osb[:])

    nc.compile()
    return nc


def _make_runner(nc, donate=True):
    """Build a cached sharded-jit runner for `nc` on 8 cores.

    Mirrors bass2jax.run_bass_via_pjrt's multi-core path, but keeps the
    jitted function so repeated calls reuse the compiled executable (the
    stock helper rebuilds the jit -> reruns the minutes-long NEFF compile
    every call). With donate=False, inputs (incl. the zero output seeds)
    can live on device and be reused across timing reps.
    """
    import jax
    import jax.numpy as jnp
    from jax.sharding import Mesh, PartitionSpec, NamedSharding
    from jax.experimental.shard_map import shard_map
    import concourse.mybir as mybir
    from concourse import bass2jax

    bass2jax.install_neuronx_cc_hook()

    partition_name = nc.partition_id_tensor.name if nc.partition_id_tensor else None
    in_names, out_names, out_avals, zero_outs = [], [], [], []
    for alloc in nc.m.functions[0].allocations:
        if not isinstance(alloc, mybir.MemoryLocationSet):
            continue
        name = alloc.memorylocations[0].name
        if alloc.kind == "ExternalInput":
            if name != partition_name:
                in_names.append(name)
        elif alloc.kind == "ExternalOutput":
            shape = tuple(alloc.tensor_shape)
            dtype = mybir.dt.np(alloc.dtype)
            out_names.append(name)
            out_avals.append(jax.core.ShapedArray(shape, dtype))
            zero_outs.append(np.zeros(shape, dtype))
    n_params = len(in_names)
    n_outs = len(out_avals)
    all_in_names = list(in_names) + list(out_names)
    if partition_name is not None:
        all_in_names.append(partition_name)

    def _body(*args):
        operands = list(args)
        if partition_name is not None:
            operands.append(bass2jax.partition_id_tensor())
        outs = bass2jax._bass_exec_p.bind(
            *operands,
            out_avals=tuple(out_avals),
            in_names=tuple(all_in_names),
            out_names=tuple(out_names),
            lowering_input_output_aliases=(),
            sim_require_finite=True,
            sim_require_nnan=True,
            nc=nc,
        )
        return tuple(outs)

    devices = jax.devices()[:N_CORES]
    mesh = Mesh(np.asarray(devices), ("core",))
    in_specs = (PartitionSpec("core"),) * (n_params + n_outs)
    out_specs = (PartitionSpec("core"),) * len(out_names)
    kwargs = dict(keep_unused=True)
    if donate:
        kwargs["donate_argnums"] = tuple(range(n_params, n_params + n_outs))
    sharded = jax.jit(
        shard_map(_body, mesh=mesh, in_specs=in_specs, out_specs=out_specs,
                  check_rep=False),
        **kwargs)
    sharding = NamedSharding(mesh, PartitionSpec("core"))

    def prep(in_maps, device_put=False):
        """concat per-core inputs (+ zero output seeds) to global arrays."""
        concat = [np.concatenate([np.asarray(m[name]) for m in in_maps], axis=0)
                  for name in in_names]
        concat += [np.concatenate([z] * N_CORES, axis=0) for z in zero_outs]
        if device_put:
            import jax
            concat = [jax.device_put(a, sharding) for a in concat]
        return concat

    def run(args):
        outs = sharded(*args)
        return outs

    def to_results(outs):
        results = [dict() for _ in range(N_CORES)]
        for name, arr in zip(out_names, outs):
            arr = np.asarray(arr)
            per = np.split(arr, N_CORES, axis=0)
            for c in range(N_CORES):
                results[c][name] = per[c]
        return results

    return prep, run, to_results


def kernel(img: np.ndarray, weight: np.ndarray, bias: np.ndarray) -> np.ndarray:
    img = np.ascontiguousarray(np.asarray(img, dtype=np.float32))
    weight = np.ascontiguousarray(np.asarray(weight, dtype=np.float32))
    bias = np.ascontiguousarray(np.asarray(bias, dtype=np.float32))

    # host-side weight/bias rearrangement (tiny): lhsT layout [C_in, tap, mt, 128]
    wt = np.ascontiguousarray(
        weight.transpose(1, 2, 3, 0).reshape(C_IN, KH * KW, MT, 128))
    bias2 = np.ascontiguousarray(bias.reshape(MT, 128).T)

    if "nc" not in _CACHED:
        _CACHED["nc"] = _build()
        _CACHED["runner"] = _make_runner(_CACHED["nc"], donate=False)
    prep, run, to_results = _CACHED["runner"]

    shards = img.reshape(N_CORES, PER_CORE, C_IN, H, W)
    in_maps = [{"img": shards[i], "wt": wt, "bias": bias2}
               for i in range(N_CORES)]

    outs = run(prep(in_maps))
    results = to_results(outs)
    _CACHED["last_results"] = results
    return np.concatenate([r["out"] for r in results], axis=0)


# revision 9
# speedup vs baseline: 1.1927x; 1.1927x over previous
"""Conv2d 3x3 (stride 1, pad 1) as 9 shifted matmuls on TRN2, data-parallel
over batch across 8 NeuronCores.

Full shapes: img [32,128,112,112] f32, weight [256,128,3,3] f32, bias [256] f32
-> out [32,256,112,112] f32.

Per core: 4 images. C_in=128 is the contraction/partition dim. The image
lives in SBUF as a zero-padded [128, 114, 114] fp32r buffer; each PSUM tile
covers 4 output rows [128, 4, 112] and accumulates 9 matmuls (one per filter
tap) reading 3D-strided slices of the padded buffer. Weights are
host-transposed to [C_in, 9, 2, 128] so lhsT tiles are direct slices.
"""

import os
import sys

sys.path.insert(0, "/opt/trn_rl_repo")

import numpy as np

N_CORES = 8
N, C_IN, H, W = 32, 128, 112, 112
C_OUT, KH, KW = 256, 3, 3
PER_CORE = N // N_CORES           # 4 images
HP, WP = H + 2, W + 2             # padded 114 x 114
RPC = 4                           # output rows per PSUM tile
NCHUNK = H // RPC                 # 28 chunks
MT = C_OUT // 128                 # 2 C_out tiles

# matmul input dtype: "f32r" (fp32 relaxed, 1 cyc/row at N>=256), "bf16",
# or "f16" (same speed as bf16, 10-bit mantissa)
MM_DTYPE = os.environ.get("CONV_MM_DTYPE", "f32r")

_CACHED = {}


def _build(repeat: int = 1):
    import contextlib
    import concourse.tile as tile
    import concourse.mybir as mybir
    from concourse import bacc

    F32 = mybir.dt.float32
    CDT = {"f32r": mybir.dt.float32r, "bf16": mybir.dt.bfloat16,
           "f16": mybir.dt.float16}[MM_DTYPE]

    nc = bacc.Bacc("TRN2", target_bir_lowering=False, debug=False)
    img_d = nc.dram_tensor("img", [PER_CORE, C_IN, H, W], F32,
                           kind="ExternalInput").ap()
    wt_d = nc.dram_tensor("wt", [C_IN, KH * KW, MT, 128], F32,
                          kind="ExternalInput").ap()
    bias_d = nc.dram_tensor("bias", [128, MT], F32, kind="ExternalInput").ap()
    out_d = nc.dram_tensor("out", [PER_CORE, C_OUT, H, W], F32,
                           kind="ExternalOutput").ap()

    with tile.TileContext(nc) as tc:
        with tc.tile_pool(name="const", bufs=1) as const_pool, \
             tc.tile_pool(name="imgpad", bufs=2) as imgpad_pool, \
             tc.tile_pool(name="ldchunk", bufs=4) as ld_pool, \
             tc.tile_pool(name="outsb", bufs=4) as out_pool, \
             tc.tile_pool(name="psum", bufs=8, space="PSUM") as psum_pool:

            # ---- constants: weights (converted to CDT) + bias ----
            wt_f32 = const_pool.tile([C_IN, KH * KW, MT, 128], F32)
            nc.sync.dma_start(wt_f32[:], wt_d[:])
            wt_c = const_pool.tile([C_IN, KH * KW, MT, 128], CDT)
            nc.vector.tensor_copy(out=wt_c[:], in_=wt_f32[:])
            bias_sb = const_pool.tile([128, MT], F32)
            nc.sync.dma_start(bias_sb[:], bias_d[:])
            zrow = const_pool.tile([C_IN, WP], F32)
            nc.vector.memset(zrow[:], 0.0)

            # repeat>1 wraps the whole body in a hardware loop for timing
            # amplification (identical work each iteration, same output).
            loop_ctx = tc.For_i(0, repeat, 1) if repeat > 1 \
                else contextlib.nullcontext()
            with loop_ctx:
              for i in range(PER_CORE):
                # ---- load + convert image i into padded CDT buffer ----
                imgp = imgpad_pool.tile([C_IN, HP, WP], CDT)
                # fp32r tiles cannot be memset; zero the pads via DVE copy
                # from an f32 zero row (a legal fp32r-rounding producer).
                nc.vector.tensor_copy(out=imgp[:, 0, :], in_=zrow[:])
                nc.vector.tensor_copy(out=imgp[:, HP - 1, :], in_=zrow[:])
                nc.vector.tensor_copy(out=imgp[:, 0:HP - 1, WP - 1],
                                      in_=zrow[:, 0:HP - 1])
                nc.vector.tensor_copy(out=imgp[:, 1:HP, 0],
                                      in_=zrow[:, 0:HP - 1])
                for c in range(NCHUNK):
                    ld = ld_pool.tile([C_IN, RPC, W], F32)
                    nc.sync.dma_start(ld[:], img_d[i, :, c * RPC:(c + 1) * RPC, :])
                    nc.vector.tensor_copy(
                        out=imgp[:, 1 + c * RPC: 1 + (c + 1) * RPC, 1:1 + W],
                        in_=ld[:])

                # ---- conv: 2 C_out tiles x 28 row-chunks x 9 taps ----
                for mt in range(MT):
                    for c in range(NCHUNK):
                        ps = psum_pool.tile([128, RPC, W], F32)
                        t = 0
                        for ky in range(KH):
                            for kx in range(KW):
                                nc.tensor.matmul(
                                    ps[:],
                                    lhsT=wt_c[:, ky * KW + kx, mt, :],
                                    rhs=imgp[:, c * RPC + ky: c * RPC + ky + RPC,
                                             kx: kx + W],
                                    start=(t == 0), stop=(t == KH * KW - 1),
                                )
                                t += 1
                        osb = out_pool.tile([128, RPC, W], F32)
                        nc.vector.tensor_scalar_add(osb[:], ps[:],
                                                    bias_sb[:, mt:mt + 1])
                        nc.sync.dma_start(
                            out_d[i, mt * 128:(mt + 1) * 128,
                                  c * RPC:(c + 1) * RPC, :],
                            osb[:])

    nc.compile()
    return nc


def _make_runner(nc, donate=True):
    """Build a cached sharded-jit runner for `nc` on 8 cores.

    Mirrors bass2jax.run_bass_via_pjrt's multi-core path, but keeps the
    jitted function so repeated calls reuse the compiled executable (the
    stock helper rebuilds the jit -> reruns the minutes-long NEFF compile
    every call). With donate=False, inputs (incl. the zero output seeds)
    can live on device and be reused across timing reps.
    """
    import jax
    import jax.numpy as jnp
    from jax.sharding import Mesh, PartitionSpec, NamedSharding
    from jax.experimental.shard_map import shard_map
    import concourse.mybir as mybir
    from concourse import bass2jax

    bass2jax.install_neuronx_cc_hook()

    partition_name = nc.partition_id_tensor.name if nc.partition_id_tensor else None
    in_names, out_names, out_avals, zero_outs = [], [], [], []
    for alloc in nc.m.functions[0].allocations:
        if not isinstance(alloc, mybir.MemoryLocationSet):
            continue
        name = alloc.memorylocations[0].name
        if alloc.kind == "ExternalInput":
            if name != partition_name:
                in_names.append(name)
        elif alloc.kind == "ExternalOutput":
            shape = tuple(alloc.tensor_shape)
            dtype = mybir.dt.np(alloc.dtype)
            out_names.append(name)
            out_avals.append(jax.core.ShapedArray(shape, dtype))
            zero_outs.append(np.zeros(shape, dtype))
    n_params = len(in_names)
    n_outs = len(out_avals)
    all_in_names = list(in_names) + list(out_names)
    if partition_name is not None:
        all_in_names.append(partition_name)

    def _body(*args):
        operands = list(args)
        if partition_name is not None:
            operands.append(bass2jax.partition_id_tensor())
        outs = bass2jax._bass_exec_p.bind(
            *operands,
            out_avals=tuple(out_avals),
            in_names=tuple(all_in_names),
            out_names=tuple(out_names),
            lowering_input_output_aliases=(),
            sim_require_finite=True,
            sim_require_nnan=True,
            nc=nc,
        )
        return tuple(outs)

    devices = jax.devices()[:N_CORES]
    mesh = Mesh(np.asarray(devices), ("core",))
    in_specs = (PartitionSpec("core"),) * (n_params + n_outs)
    out_specs = (PartitionSpec("core"),) * len(out_names)
    kwargs = dict(keep_unused=True)
    if donate:
        kwargs["donate_argnums"] = tuple(range(n_params, n_params + n_outs))
    sharded = jax.jit(
        shard_map(_body, mesh=mesh, in_specs=in_specs, out_specs=out_specs,
                  check_rep=False),
        **kwargs)
    sharding = NamedSharding(mesh, PartitionSpec("core"))

    def prep(in_maps, device_put=False):
        """concat per-core inputs (+ zero output seeds) to global arrays."""
        concat = [np.concatenate([np.asarray(m[name]) for m in in_maps], axis=0)
                  for name in in_names]
        concat += [np.concatenate([z] * N_CORES, axis=0) for z in zero_outs]
        if device_put:
            import jax
            concat = [jax.device_put(a, sharding) for a in concat]
        return concat

    def run(args):
        outs = sharded(*args)
        return outs

    def to_results(outs):
        results = [dict() for _ in range(N_CORES)]
        for name, arr in zip(out_names, outs):
            arr = np.asarray(arr)
            per = np.split(arr, N_CORES, axis=0)
            for c in range(N_CORES):
                results[c][name] = per[c]
        return results

    return prep, run, to_results


def kernel(img: np.ndarray, weight: np.ndarray, bias: np.ndarray) -> np.ndarray:
    img = np.ascontiguousarray(np.asarray(img, dtype=np.float32))
    weight = np.ascontiguousarray(np.asarray(weight, dtype=np.float32))
    bias = np.ascontiguousarray(np.asarray(bias, dtype=np.float32))

    # host-side weight/bias rearrangement (tiny): lhsT layout [C_in, tap, mt, 128]
    wt = np.ascontiguousarray(
        weight.transpose(1, 2, 3, 0).reshape(C_IN, KH * KW, MT, 128))
    bias2 = np.ascontiguousarray(bias.reshape(MT, 128).T)

    if "nc" not in _CACHED:
        _CACHED["nc"] = _build()
        _CACHED["runner"] = _make_runner(_CACHED["nc"], donate=False)
    prep, run, to_results = _CACHED["runner"]

    shards = img.reshape(N_CORES, PER_CORE, C_IN, H, W)
    in_maps = [{"img": shards[i], "wt": wt, "bias": bias2}
               for i in range(N_CORES)]

    outs = run(prep(in_maps))
    results = to_results(outs)
    _CACHED["last_results"] = results
    return np.concatenate([r["out"] for r in results], axis=0)
